# revision 11
# baseline (speedup 1.0000x reference)
"""DotProductDistributionHead kernel for Trainium2 (Bass/Tile), 8-core data-parallel.

Computation (per reference):
    h = gelu(x @ W_mu + b_mu)            # (B, D) with erf gelu
    logits[b, n] = h[b] . emb_table[candidates[b, n]] + mu_bias[candidates[b, n]]

Sharding: x/candidates split along batch across 8 cores; W_mu / b_mu /
emb_table replicated per core (each core's copy lands in its own HBM
stack, so gather bandwidth scales with cores).

Device layout per core (B_LOC = 512 rows):
  - h computed as 4 PE matmuls (x^T pre-transposed on host), erf-Gelu on ACT.
  - emb rows fetched with gpsimd indirect DMA gather: index tile [128, S]
    (partition p = batch row within the 128-row block, slot s = candidate)
    pulls 512B rows into G [128, S*128] (row (p,s) contiguous at [p, s*128:]).
  - dot: per chunk, DVE multiply G by h (broadcast along the S axis) and a
    segmented reduce over the inner 128 elements -> logits [128, S].
  - mu_bias is all-zeros per the problem spec; a host-side fallback adds it
    if a nonzero vector is ever passed.
"""

import numpy as np

import concourse.bacc as bacc
import concourse.bass as bass
import concourse.tile as tile
from concourse import mybir
from concourse.bass_utils import run_bass_kernel_spmd

B, N, D, V = 4096, 200, 128, 100000
NCORES = 8
B_LOC = B // NCORES          # 512 batch rows per core
NBLK = B_LOC // 128          # 4 blocks of 128 rows
S = 50                       # candidate-slots per gather chunk
N_CHUNKS = N // S

USE_SCAN = False             # custom DVE fused mul+scan (faster path)
TRACE = False                # set by test.py to capture an NTFF profile
LAST_RESULTS = None
ACT_FUNC = "Gelu"            # overridden in sim-debug (CoreSim lacks Gelu)

_f32 = mybir.dt.float32
_i32 = mybir.dt.int32

_program_cache = {}


CONST_COLS = D + B_LOC + D  # [W | xT | b_mu replicated]


def _kernel_body(tc, consts, cand, emb, out):
    nc = tc.nc
    gelu = getattr(mybir.ActivationFunctionType, ACT_FUNC)

    with (
        tc.tile_pool(name="const", bufs=1) as cpool,
        tc.tile_pool(name="psum", bufs=2, space="PSUM") as ppool,
        tc.tile_pool(name="cands", bufs=2) as candpool,
        tc.tile_pool(name="outs", bufs=2) as outpool,
        tc.tile_pool(name="gather", bufs=3) as gpool,
        tc.tile_pool(name="scratch", bufs=2) as spool,
    ):
        c_sb = cpool.tile([128, CONST_COLS], _f32)
        nc.sync.dma_start(c_sb[:], consts[:, :])
        W_sb = c_sb[:, 0:D]
        xT_sb = c_sb[:, D : D + B_LOC]
        bias_sb = c_sb[:, D + B_LOC : D + B_LOC + D]

        # h[b, d] for all 512 local rows: block c lives at h_sb[:, c*D:(c+1)*D]
        h_sb = cpool.tile([128, NBLK * D], _f32)
        for c in range(NBLK):
            ps = ppool.tile([128, D], _f32)
            nc.tensor.matmul(
                out=ps[:], lhsT=xT_sb[:, c * 128 : (c + 1) * 128], rhs=W_sb,
                start=True, stop=True,
            )
            nc.vector.tensor_tensor(
                out=ps[:], in0=ps[:], in1=bias_sb, op=mybir.AluOpType.add
            )
            nc.scalar.activation(out=h_sb[:, c * D : (c + 1) * D], in_=ps[:], func=gelu)

        for c in range(NBLK):
            cand_sb = candpool.tile([128, N], _i32)
            nc.sync.dma_start(cand_sb[:], cand[c * 128 : (c + 1) * 128, :])
            logits_sb = outpool.tile([128, N], _f32)
            h_blk = h_sb[:, c * D : (c + 1) * D]
            for s in range(N_CHUNKS):
                G = gpool.tile([128, S * D], _f32)
                for j in range(S):
                    nc.gpsimd.indirect_dma_start(
                        out=G[:, j * D : (j + 1) * D],
                        out_offset=None,
                        in_=emb[:, :],
                        in_offset=bass.IndirectOffsetOnAxis(
                            ap=cand_sb[:, s * S + j : s * S + j + 1], axis=0
                        ),
                    )
                G3 = G[:].rearrange("p (s d) -> p s d", d=D)
                h_bc = h_blk.unsqueeze(1).to_broadcast([128, S, D])
                prod = spool.tile([128, S * D], _f32)
                nc.vector.tensor_tensor(
                    out=prod[:].rearrange("p (s d) -> p s d", d=D),
                    in0=G3, in1=h_bc, op=mybir.AluOpType.mult,
                )
                nc.vector.tensor_reduce(
                    out=logits_sb[:, s * S : (s + 1) * S],
                    in_=prod[:].rearrange("p (s d) -> p s d", d=D),
                    axis=mybir.AxisListType.X,
                    op=mybir.AluOpType.add,
                )
            nc.sync.dma_start(out[c * 128 : (c + 1) * 128, :], logits_sb[:])


def _build_program():
    key = (USE_SCAN, S)
    if key in _program_cache:
        return _program_cache[key]
    nc = bacc.Bacc(
        "TRN2",
        target_bir_lowering=False,
        debug=False,
        enable_asserts=False,
        num_devices=NCORES,
    )
    consts = nc.dram_tensor("consts", (128, CONST_COLS), _f32, kind="ExternalInput").ap()
    cand = nc.dram_tensor("cand", (B_LOC, N), _i32, kind="ExternalInput").ap()
    emb = nc.dram_tensor("emb", (V, D), _f32, kind="ExternalInput").ap()
    out = nc.dram_tensor("out", (B_LOC, N), _f32, kind="ExternalOutput").ap()
    with tile.TileContext(nc) as tc:
        _kernel_body(tc, consts, cand, emb, out)
    nc.finalize()
    _program_cache[key] = nc
    return nc


def kernel(x, candidates, W_mu, b_mu, mu_bias, emb_table):
    global LAST_RESULTS
    x = np.asarray(x, dtype=np.float32)
    candidates = np.asarray(candidates)
    W_mu = np.ascontiguousarray(np.asarray(W_mu, dtype=np.float32))
    b_mu = np.asarray(b_mu, dtype=np.float32)
    mu_bias = np.asarray(mu_bias, dtype=np.float32)
    emb = np.ascontiguousarray(np.asarray(emb_table, dtype=np.float32))

    nc = _build_program()
    bias_tile = np.broadcast_to(b_mu.reshape(1, D), (128, D))
    in_maps = []
    for c in range(NCORES):
        sl = slice(c * B_LOC, (c + 1) * B_LOC)
        consts = np.concatenate([W_mu, x[sl].T, bias_tile], axis=1)
        in_maps.append(
            {
                "consts": np.ascontiguousarray(consts, dtype=np.float32),
                "cand": np.ascontiguousarray(candidates[sl].astype(np.int32)),
                "emb": emb,
            }
        )
    res = run_bass_kernel_spmd(
        nc, in_maps, core_ids=list(range(NCORES)), trace=TRACE
    )
    LAST_RESULTS = res
    logits = np.concatenate([r["out"] for r in res.results], axis=0)
    if np.any(mu_bias):
        logits = logits + mu_bias[candidates]
    return np.ascontiguousarray(logits.astype(np.float32))
